# revision 32
# baseline (speedup 1.0000x reference)
"""Supervised contrastive loss on 8 Trainium2 NeuronCores.

Data-parallel over embedding rows (512 rows/core), label-sorted so each
128-row m-tile's same-label partners live in one <=256-column window.

Per core, per m-tile (ACT is the bottleneck engine; structure minimizes
ACT element count and instruction count):
  - window: 2 fp8 DoubleRow matmuls recompute the [128, 256] same-label
    sims; all 4 m-tiles' windows share one [128, 1024] PSUM tile drained
    by ONE Exp activate. Mask (label equality, self poisoned to -2) and
    masked row-sums run on DVE -> ssame (diagonal excluded).
  - dense: k-outer matmuls fill [128, 2048] PSUM halves (ping-pong);
    ONE Exp activate per half writes an fp16 tile; DVE row-reduces it
    (fp16 keeps the e^{s_ii}~e^10 diagonal accurate enough to subtract).
  - per-row loss, with cnt, e^{s_ii} and sum_j s_ij host-precomputed:
      denom = sall - ssame - e_ii ;  lnden = ln(denom)
      rowtot = (cnt-1)*lnden + sum_win ln(1 + mew/denom)
    (the host adds sum_rows (s_ii - sum_same s_ij) and divides by the
    exact positive count).
"""

import ml_dtypes
import numpy as np

import concourse.bass as bass
import concourse.bacc as bacc
import concourse.mybir as mybir
import concourse.tile as tile
from concourse.bass_utils import run_bass_kernel_spmd
from concourse.hw_specs import get_activation_tables

B = 4096          # total rows
D = 512           # embedding dim
NCORES = 8
BL = B // NCORES  # rows per core
NKK = 2           # DoubleRow k-tiles (256 contraction rows each)
NMT = BL // 128   # m-tiles per core
HALF = 2048       # dense column half (4 PSUM banks)
WIN = 256         # same-label column window per m-tile (data max is 223)
MMW = 512        # matmul output width (columns per matmul instruction)
TINV = 10.0       # 1 / temperature
F32 = mybir.dt.float32
F16 = mybir.dt.float16
BF16 = mybir.dt.bfloat16
F8 = mybir.dt.float8e4
NP_F8 = mybir.dt.np(F8)
SCALE = 16.0      # fp8 pre-scale; folded out via the Exp activation scale
ESC = TINV / (SCALE * SCALE)

_CACHE = {}


def _build_nc():
    nc = bacc.Bacc()
    # et packed as 4 blocks (k0,h0),(k1,h0),(k0,h1),(k1,h1): k-split lets the
    # k0 matmuls of a half start before its k1 block lands
    et = nc.dram_tensor("et", [NKK * 2, 128, 2, HALF], F8, kind="ExternalInput")
    # pack8: one full-line DMA for all small fp8 inputs.
    # j=0,1: own-row lhsT k-tiles; j=2..5: window rhs for m-tile j-2
    pack8 = nc.dram_tensor("pack8", [128, 2 + NMT, 2, 2, WIN], F8,
                           kind="ExternalInput")
    # pack16: window labels (self poisoned -2, pad -1) + per-row scalars
    # [..., WIN]=row label, [WIN+1]=cnt-1, [WIN+2]=eii_hi, [WIN+3]=eii_lo
    pack16 = nc.dram_tensor("pack16", [128, NMT, WIN + 4], BF16,
                            kind="ExternalInput")
    out = nc.dram_tensor("out", [128, NMT], F32, kind="ExternalOutput")

    AF = mybir.ActivationFunctionType
    OP = mybir.AluOpType
    DR = mybir.MatmulPerfMode.DoubleRow
    AX = mybir.AxisListType.X

    with tile.TileContext(nc) as tc:
        with (
            tc.tile_pool(name="const", bufs=1) as cpool,
            tc.tile_pool(name="psum", bufs=2, space=bass.MemorySpace.PSUM) as ppool,
            tc.tile_pool(name="expo", bufs=2) as epool,
            tc.tile_pool(name="win", bufs=1) as wpool,
            tc.tile_pool(name="small", bufs=1) as spool,
        ):
            # [128, half, kk2, 2, HALF]: each block DMA is one contiguous
            # 4 KiB line per partition
            etall = cpool.tile([128, 2, NKK, 2, HALF], F8, tag="etall",
                               name="etall")
            pk8 = cpool.tile([128, 2 + NMT, 2, 2, WIN], F8, tag="pk8",
                             name="pk8")
            pk16 = cpool.tile([128, NMT, WIN + 4], BF16, tag="pk16",
                              name="pk16")

            def lhsT(k, mt):
                # own-row weights for m-tile mt: [128, 2, 128]
                return pk8[:, k, :, mt // 2,
                           (mt % 2) * 128:(mt % 2) * 128 + 128]

            def rhs(k, h, c, w):
                # dense columns [h*HALF+c, +w): [128, 2, w]
                return etall[:, h, k, :, c:c + w]

            # DMA issue: big et blocks on the Act HWDGE ring (issued before
            # any activation work), packed small inputs on the SP ring.
            for j in range(2):
                nc.scalar.dma_start(etall[:, j // 2, j % 2], et[j])
            # one table load serves both Exp and Ln (avoids set thrash)
            tabs = list(get_activation_tables(nc.m.arch).keys())
            nc.scalar.add_instruction(mybir.InstLoadActFuncSet(
                name=nc.get_next_instruction_name(), ins=[], outs=[],
                act_func_set_id=tabs.index("natural_log_exp_and_others")))
            for j in range(2, NKK * 2):
                nc.scalar.dma_start(etall[:, j // 2, j % 2], et[j])
            nc.sync.dma_start(pk8[:], pack8[:])
            nc.sync.dma_start(pk16[:], pack16[:])

            # ---- window sims: one PSUM tile, one Exp ----
            wp = ppool.tile([128, HALF], F32, tag="ps", name="wpsum")
            for mt in range(NMT):
                for k in range(NKK):
                    nc.tensor.matmul(
                        wp[:, mt * WIN:(mt + 1) * WIN],
                        lhsT(k, mt), pk8[:, 2 + mt, k],
                        start=(k == 0), stop=(k == NKK - 1), perf_mode=DR)
            expw = wpool.tile([128, NMT, WIN], BF16, tag="expw", name="expw")
            nc.scalar.activation(expw[:], wp[:, 0:NMT * WIN], AF.Exp, scale=ESC)

            # per-row scalars to f32: [...,0]=row label, [...,1]=cnt-1
            scal = spool.tile([128, NMT, 2], F32, tag="scal", name="scal")
            nc.vector.tensor_scalar(scal[:], pk16[:, :, WIN:WIN + 2], 0.0,
                                    None, OP.add)
            # e^{s_ii} from its two bf16 halves
            eii = spool.tile([128, NMT], F32, tag="eii", name="eii")
            nc.vector.tensor_reduce(eii[:], pk16[:, :, WIN + 2:WIN + 4],
                                    AX, OP.add)

            maskw = wpool.tile([128, NMT, WIN], BF16, tag="maskw", name="maskw")
            for mt in range(NMT):
                nc.vector.tensor_scalar(
                    maskw[:, mt], pk16[:, mt, 0:WIN],
                    scal[:, mt, 0:1], None, OP.is_equal)
            mew = wpool.tile([128, NMT, WIN], BF16, tag="mew", name="mew")
            nc.vector.tensor_tensor(mew[:], expw[:], maskw[:], OP.mult)
            ssame = spool.tile([128, NMT], F32, tag="ssame", name="ssame")
            nc.vector.tensor_reduce(ssame[:], mew[:], AX, OP.add)

            # ---- dense + per-m-tile tail ----
            rsum = spool.tile([128, NMT, 2], F32, tag="rsum", name="rsum")
            sall = spool.tile([128, NMT], F32, tag="sall", name="sall")
            denom = spool.tile([128, NMT], F32, tag="denom", name="denom")
            inv = spool.tile([128, NMT], F32, tag="inv", name="inv")
            lnden = spool.tile([128, NMT], F32, tag="lnden", name="lnden")
            slog = spool.tile([128, NMT], F32, tag="slog", name="slog")
            rowt = spool.tile([128, NMT], F32, tag="rowt", name="rowt")
            mewi = wpool.tile([128, NMT, WIN], BF16, tag="mewi", name="mewi")
            lnp = wpool.tile([128, NMT, WIN], BF16, tag="lnp", name="lnp")

            for mt in range(NMT):
                for h in range(2):
                    P = ppool.tile([128, HALF], F32, tag="ps",
                                   name=f"ps{mt}_{h}")
                    for k in range(NKK):
                        w = lhsT(k, mt)
                        for j in range(HALF // MMW):
                            nc.tensor.matmul(
                                P[:, j * MMW:(j + 1) * MMW],
                                w, rhs(k, h, j * MMW, MMW),
                                start=(k == 0), stop=(k == NKK - 1),
                                perf_mode=DR)
                    if h == 0:
                        # h0 row-sum on DVE (fp16 keeps the e^{s_ii} diagonal
                        # accurate); hidden under the h1 chunk's work
                        E = epool.tile([128, HALF], F16, tag="expA",
                                       name=f"exp{mt}_{h}")
                        nc.scalar.activation(E[:], P[:], AF.Exp, scale=ESC)
                        nc.vector.tensor_reduce(rsum[:, mt, 0:1], E[:],
                                                AX, OP.add)
                    elif mt < 2:
                        # early m-tiles: h1 row-sum on DVE as well; the tail
                        # latency hides under later m-tiles' dense work
                        E = epool.tile([128, HALF], F16, tag="expA",
                                       name=f"exp{mt}_{h}")
                        nc.scalar.activation(E[:], P[:], AF.Exp, scale=ESC)
                        nc.vector.tensor_reduce(rsum[:, mt, 1:2], E[:],
                                                AX, OP.add)
                    else:
                        E = epool.tile([128, HALF], BF16, tag="expB",
                                       name=f"exp{mt}_{h}")
                        nc.scalar.activation(E[:], P[:], AF.Exp, scale=ESC,
                                             accum_out=rsum[:, mt, 1:2])
                # tail for this m-tile (overlaps later m-tiles' dense work)
                nc.vector.tensor_reduce(sall[:, mt:mt + 1], rsum[:, mt],
                                        AX, OP.add)
                nc.vector.tensor_scalar(
                    denom[:, mt:mt + 1], sall[:, mt:mt + 1],
                    ssame[:, mt:mt + 1], eii[:, mt:mt + 1],
                    OP.subtract, OP.subtract)
                nc.vector.reciprocal(inv[:, mt:mt + 1], denom[:, mt:mt + 1])
                nc.scalar.activation(lnden[:, mt:mt + 1], denom[:, mt:mt + 1],
                                     AF.Ln)
                nc.vector.tensor_scalar(mewi[:, mt], mew[:, mt],
                                        inv[:, mt:mt + 1], None, OP.mult)
                nc.scalar.activation(lnp[:, mt], mewi[:, mt], AF.Ln,
                                     scale=1.0, bias=1.0)
                nc.vector.tensor_reduce(slog[:, mt:mt + 1], lnp[:, mt],
                                        AX, OP.add)
                nc.vector.tensor_scalar(
                    rowt[:, mt:mt + 1], lnden[:, mt:mt + 1],
                    scal[:, mt, 1:2], slog[:, mt:mt + 1],
                    OP.mult, OP.add)
                nc.sync.dma_start(out[:, mt:mt + 1], rowt[:, mt:mt + 1])
    nc.compile()
    return nc


def _make_in_maps(embeddings, labels):
    """Host prep: label-sort, fp8 transposes, windows, per-row scalars.
    Returns (in_maps, num_pos, host_extra): host_extra is added to the
    device row-total sum before dividing by num_pos."""
    emb0 = np.ascontiguousarray(np.asarray(embeddings, dtype=np.float32))
    lab0 = np.asarray(labels).astype(np.int64)
    assert emb0.shape == (B, D) and lab0.shape == (B,)

    perm = np.argsort(lab0, kind="stable")
    emb = emb0[perm]
    lab = lab0[perm]

    ET = np.ascontiguousarray(emb.T)                      # [D, B] sorted cols
    ET8 = (ET * SCALE).astype(NP_F8)

    def dr_pack(a):
        # [D, X] -> [NKK, 128, 2, X] with d = kk*256 + ko*128 + ki
        X = a.shape[1]
        return np.ascontiguousarray(
            a.reshape(NKK, 2, 128, X).transpose(0, 2, 1, 3))

    lab16 = lab.astype(np.float32).astype(ml_dtypes.bfloat16)

    ncls = int(lab.max()) + 1
    counts = np.bincount(lab, minlength=ncls)
    cum = np.concatenate([[0], np.cumsum(counts)])
    cnt = counts[lab].astype(np.float64)                  # incl. self
    num_pos = float(cnt.sum() - B)

    emb64 = emb.astype(np.float64)
    G = np.zeros((ncls, D), np.float64)
    np.add.at(G, lab, emb64)
    rds = (emb64 * G[lab]).sum(1) * TINV                  # sum_same s_ij (incl self)
    sii = (emb64 * emb64).sum(1) * TINV
    host_extra = float((sii - rds).sum())                 # sum_rows (s_ii - sum_{j!=i} s_ij)

    # device-exact e^{s_ii}: replicate the fp8 matmul's diagonal
    et64 = ET8.astype(np.float64)
    sii_dev = (et64 * et64).sum(0) * ESC                  # [B]
    eii_dev = np.exp(sii_dev).astype(np.float32)

    dr = dr_pack(ET8)                                     # [NKK, 128, 2, B]
    et_h = np.ascontiguousarray(np.stack([
        dr[0][:, :, 0:HALF], dr[1][:, :, 0:HALF],
        dr[0][:, :, HALF:], dr[1][:, :, HALF:]]))         # [4, 128, 2, HALF]

    bf = ml_dtypes.bfloat16
    in_maps = []
    for c in range(NCORES):
        sl = slice(c * BL, (c + 1) * BL)
        etwin = np.zeros((NMT, D, WIN), NP_F8)
        pack16 = np.zeros((128, NMT, WIN + 4), bf)
        pack16[:, :, :WIN] = bf(-1.0)                     # pad: matches no label
        for m in range(NMT):
            r0 = c * BL + m * 128
            c0 = int(cum[lab[r0]])
            c1 = int(cum[lab[r0 + 127] + 1])
            w = c1 - c0
            assert w <= WIN, f"window {w} exceeds {WIN}; rebuild with larger WIN"
            etwin[m, :, :w] = ET8[:, c0:c1]
            pack16[:, m, :w] = lab16[c0:c1][None, :]
            for p in range(128):
                pack16[p, m, r0 + p - c0] = bf(-2.0)      # poison self
            pack16[:, m, WIN] = lab16[r0:r0 + 128]
            pack16[:, m, WIN + 1] = (cnt[r0:r0 + 128] - 1.0).astype(bf)
            ehi = eii_dev[r0:r0 + 128].astype(bf)
            pack16[:, m, WIN + 2] = ehi
            pack16[:, m, WIN + 3] = (
                eii_dev[r0:r0 + 128] - ehi.astype(np.float32)).astype(bf)
        # pack8: [128, 2+NMT, 2, 2, WIN]; j=0,1 own-row lhsT; j=2+m window rhs
        pack8 = np.zeros((128, 2 + NMT, 2, 2, WIN), NP_F8)
        eltp = dr_pack(np.ascontiguousarray(ET8[:, sl]))  # [NKK, 128, 2, BL]
        for k in range(NKK):
            pack8[:, k] = eltp[k].reshape(128, 2, 2, WIN)
        etwin_packed = etwin.reshape(NMT, NKK, 2, 128, WIN).transpose(
            0, 3, 1, 2, 4)                                # [NMT, 128, 2, 2, WIN]
        for m in range(NMT):
            pack8[:, 2 + m] = etwin_packed[m]
        in_maps.append({
            "et": et_h,
            "pack8": pack8,
            "pack16": pack16,
        })
    return in_maps, num_pos, host_extra


def kernel(embeddings, labels):
    in_maps, num_pos, host_extra = _make_in_maps(embeddings, labels)
    if "nc" not in _CACHE:
        _CACHE["nc"] = _build_nc()
    nc = _CACHE["nc"]
    res = run_bass_kernel_spmd(nc, in_maps, list(range(NCORES)))
    total = sum(float(r["out"].sum()) for r in res.results) + host_extra
    return np.asarray(total / max(num_pos, 1.0), dtype=np.float32)


# revision 33
# speedup vs baseline: 1.0661x; 1.0661x over previous
"""Supervised contrastive loss on 8 Trainium2 NeuronCores.

Data-parallel over embedding rows (512 rows/core), label-sorted so each
128-row m-tile's same-label partners live in one <=256-column window.

Per core, per m-tile (ACT is the bottleneck engine; structure minimizes
ACT element count and instruction count):
  - window: 2 fp8 DoubleRow matmuls recompute the [128, 256] same-label
    sims; all 4 m-tiles' windows share one [128, 1024] PSUM tile drained
    by ONE Exp activate. Mask (label equality, self poisoned to -2) and
    masked row-sums run on DVE -> ssame (diagonal excluded).
  - dense: k-outer matmuls fill [128, 2048] PSUM halves (ping-pong);
    ONE Exp activate per half writes an fp16 tile; DVE row-reduces it
    (fp16 keeps the e^{s_ii}~e^10 diagonal accurate enough to subtract).
  - per-row loss, with cnt, e^{s_ii} and sum_j s_ij host-precomputed:
      denom = sall - ssame - e_ii ;  lnden = ln(denom)
      rowtot = (cnt-1)*lnden + sum_win ln(1 + mew/denom)
    (the host adds sum_rows (s_ii - sum_same s_ij) and divides by the
    exact positive count).
"""

import ml_dtypes
import numpy as np

import concourse.bass as bass
import concourse.bacc as bacc
import concourse.mybir as mybir
import concourse.tile as tile
from concourse.bass_utils import run_bass_kernel_spmd
from concourse.hw_specs import get_activation_tables

B = 4096          # total rows
D = 512           # embedding dim
NCORES = 8
BL = B // NCORES  # rows per core
NKK = 2           # DoubleRow k-tiles (256 contraction rows each)
NMT = BL // 128   # m-tiles per core
HALF = 2048       # dense column half (4 PSUM banks)
WIN = 256         # same-label column window per m-tile (data max is 223)
MMW = 512        # matmul output width (columns per matmul instruction)
TINV = 10.0       # 1 / temperature
F32 = mybir.dt.float32
F16 = mybir.dt.float16
BF16 = mybir.dt.bfloat16
F8 = mybir.dt.float8e4
NP_F8 = mybir.dt.np(F8)
SCALE = 16.0      # fp8 pre-scale; folded out via the Exp activation scale
ESC = TINV / (SCALE * SCALE)

_CACHE = {}


def _build_nc():
    nc = bacc.Bacc()
    # et packed as 4 blocks (k0,h0),(k1,h0),(k0,h1),(k1,h1): k-split lets the
    # k0 matmuls of a half start before its k1 block lands
    et = nc.dram_tensor("et", [NKK * 2, 128, 2, HALF], F8, kind="ExternalInput")
    # pack8: one full-line DMA for all small fp8 inputs.
    # j=0,1: own-row lhsT k-tiles; j=2..5: window rhs for m-tile j-2
    pack8 = nc.dram_tensor("pack8", [128, 2 + NMT, 2, 2, WIN], F8,
                           kind="ExternalInput")
    # pack16: window labels (self poisoned -2, pad -1) + per-row scalars
    # [..., WIN]=row label, [WIN+1]=cnt-1, [WIN+2]=eii_hi, [WIN+3]=eii_lo
    pack16 = nc.dram_tensor("pack16", [128, NMT, WIN + 4], BF16,
                            kind="ExternalInput")
    out = nc.dram_tensor("out", [128, NMT], F32, kind="ExternalOutput")

    AF = mybir.ActivationFunctionType
    OP = mybir.AluOpType
    DR = mybir.MatmulPerfMode.DoubleRow
    AX = mybir.AxisListType.X

    with tile.TileContext(nc) as tc:
        with (
            tc.tile_pool(name="const", bufs=1) as cpool,
            tc.tile_pool(name="psum", bufs=2, space=bass.MemorySpace.PSUM) as ppool,
            tc.tile_pool(name="expo", bufs=2) as epool,
            tc.tile_pool(name="win", bufs=1) as wpool,
            tc.tile_pool(name="small", bufs=1) as spool,
        ):
            # [128, half, kk2, 2, HALF]: each block DMA is one contiguous
            # 4 KiB line per partition
            etall = cpool.tile([128, 2, NKK, 2, HALF], F8, tag="etall",
                               name="etall")
            pk8 = cpool.tile([128, 2 + NMT, 2, 2, WIN], F8, tag="pk8",
                             name="pk8")
            pk16 = cpool.tile([128, NMT, WIN + 4], BF16, tag="pk16",
                              name="pk16")

            def lhsT(k, mt):
                # own-row weights for m-tile mt: [128, 2, 128]
                return pk8[:, k, :, mt // 2,
                           (mt % 2) * 128:(mt % 2) * 128 + 128]

            def rhs(k, h, c, w):
                # dense columns [h*HALF+c, +w): [128, 2, w]
                return etall[:, h, k, :, c:c + w]

            # DMA issue: big et blocks on the Act HWDGE ring (issued before
            # any activation work), packed small inputs on the SP ring.
            for j in range(2):
                nc.scalar.dma_start(etall[:, j // 2, j % 2], et[j])
            # one table load serves both Exp and Ln (avoids set thrash)
            tabs = list(get_activation_tables(nc.m.arch).keys())
            nc.scalar.add_instruction(mybir.InstLoadActFuncSet(
                name=nc.get_next_instruction_name(), ins=[], outs=[],
                act_func_set_id=tabs.index("natural_log_exp_and_others")))
            for j in range(2, NKK * 2):
                nc.scalar.dma_start(etall[:, j // 2, j % 2], et[j])
            nc.sync.dma_start(pk8[:], pack8[:])
            nc.sync.dma_start(pk16[:], pack16[:])

            # ---- window sims: one PSUM tile, one Exp ----
            wp = ppool.tile([128, HALF], F32, tag="ps", name="wpsum")
            for mt in range(NMT):
                for k in range(NKK):
                    nc.tensor.matmul(
                        wp[:, mt * WIN:(mt + 1) * WIN],
                        lhsT(k, mt), pk8[:, 2 + mt, k],
                        start=(k == 0), stop=(k == NKK - 1), perf_mode=DR)
            expw = wpool.tile([128, NMT, WIN], BF16, tag="expw", name="expw")
            nc.scalar.activation(expw[:], wp[:, 0:NMT * WIN], AF.Exp, scale=ESC)

            # per-row scalars to f32: [...,0]=row label, [...,1]=cnt-1
            scal = spool.tile([128, NMT, 2], F32, tag="scal", name="scal")
            nc.vector.tensor_scalar(scal[:], pk16[:, :, WIN:WIN + 2], 0.0,
                                    None, OP.add)
            # e^{s_ii} from its two bf16 halves
            eii = spool.tile([128, NMT], F32, tag="eii", name="eii")
            nc.vector.tensor_reduce(eii[:], pk16[:, :, WIN + 2:WIN + 4],
                                    AX, OP.add)

            maskw = wpool.tile([128, NMT, WIN], BF16, tag="maskw", name="maskw")
            for mt in range(NMT):
                nc.vector.tensor_scalar(
                    maskw[:, mt], pk16[:, mt, 0:WIN],
                    scal[:, mt, 0:1], None, OP.is_equal)
            mew = wpool.tile([128, NMT, WIN], BF16, tag="mew", name="mew")
            nc.vector.tensor_tensor(mew[:], expw[:], maskw[:], OP.mult)
            ssame = spool.tile([128, NMT], F32, tag="ssame", name="ssame")
            nc.vector.tensor_reduce(ssame[:], mew[:], AX, OP.add)

            # ---- dense + per-m-tile tail ----
            rsum = spool.tile([128, NMT, 2], F32, tag="rsum", name="rsum")
            sall = spool.tile([128, NMT], F32, tag="sall", name="sall")
            denom = spool.tile([128, NMT], F32, tag="denom", name="denom")
            inv = spool.tile([128, NMT], F32, tag="inv", name="inv")
            lnden = spool.tile([128, NMT], F32, tag="lnden", name="lnden")
            slog = spool.tile([128, NMT], F32, tag="slog", name="slog")
            rowt = spool.tile([128, NMT], F32, tag="rowt", name="rowt")
            mewi = wpool.tile([128, NMT, WIN], BF16, tag="mewi", name="mewi")
            lnp = wpool.tile([128, NMT, WIN], BF16, tag="lnp", name="lnp")

            for mt in range(NMT):
                for h in range(2):
                    P = ppool.tile([128, HALF], F32, tag="ps",
                                   name=f"ps{mt}_{h}")
                    for k in range(NKK):
                        w = lhsT(k, mt)
                        for j in range(HALF // MMW):
                            nc.tensor.matmul(
                                P[:, j * MMW:(j + 1) * MMW],
                                w, rhs(k, h, j * MMW, MMW),
                                start=(k == 0), stop=(k == NKK - 1),
                                perf_mode=DR)
                    if h == 0:
                        # h0 row-sum on DVE (fp16 keeps the e^{s_ii} diagonal
                        # accurate); hidden under the h1 chunk's work
                        E = epool.tile([128, HALF], F16, tag="expA",
                                       name=f"exp{mt}_{h}")
                        nc.scalar.activation(E[:], P[:], AF.Exp, scale=ESC)
                        nc.vector.tensor_reduce(rsum[:, mt, 0:1], E[:],
                                                AX, OP.add)
                    else:
                        E = epool.tile([128, HALF], BF16, tag="expB",
                                       name=f"exp{mt}_{h}")
                        nc.scalar.activation(E[:], P[:], AF.Exp, scale=ESC,
                                             accum_out=rsum[:, mt, 1:2])
                # tail for this m-tile (overlaps later m-tiles' dense work)
                nc.vector.tensor_reduce(sall[:, mt:mt + 1], rsum[:, mt],
                                        AX, OP.add)
                nc.vector.tensor_scalar(
                    denom[:, mt:mt + 1], sall[:, mt:mt + 1],
                    ssame[:, mt:mt + 1], eii[:, mt:mt + 1],
                    OP.subtract, OP.subtract)
                nc.vector.reciprocal(inv[:, mt:mt + 1], denom[:, mt:mt + 1])
                nc.scalar.activation(lnden[:, mt:mt + 1], denom[:, mt:mt + 1],
                                     AF.Ln)
                nc.vector.tensor_scalar(mewi[:, mt], mew[:, mt],
                                        inv[:, mt:mt + 1], None, OP.mult)
                nc.scalar.activation(lnp[:, mt], mewi[:, mt], AF.Ln,
                                     scale=1.0, bias=1.0)
                nc.vector.tensor_reduce(slog[:, mt:mt + 1], lnp[:, mt],
                                        AX, OP.add)
                nc.vector.tensor_scalar(
                    rowt[:, mt:mt + 1], lnden[:, mt:mt + 1],
                    scal[:, mt, 1:2], slog[:, mt:mt + 1],
                    OP.mult, OP.add)
            nc.scalar.dma_start(out[:], rowt[:])
    nc.compile()
    return nc


def _make_in_maps(embeddings, labels):
    """Host prep: label-sort, fp8 transposes, windows, per-row scalars.
    Returns (in_maps, num_pos, host_extra): host_extra is added to the
    device row-total sum before dividing by num_pos."""
    emb0 = np.ascontiguousarray(np.asarray(embeddings, dtype=np.float32))
    lab0 = np.asarray(labels).astype(np.int64)
    assert emb0.shape == (B, D) and lab0.shape == (B,)

    perm = np.argsort(lab0, kind="stable")
    emb = emb0[perm]
    lab = lab0[perm]

    ET = np.ascontiguousarray(emb.T)                      # [D, B] sorted cols
    ET8 = (ET * SCALE).astype(NP_F8)

    def dr_pack(a):
        # [D, X] -> [NKK, 128, 2, X] with d = kk*256 + ko*128 + ki
        X = a.shape[1]
        return np.ascontiguousarray(
            a.reshape(NKK, 2, 128, X).transpose(0, 2, 1, 3))

    lab16 = lab.astype(np.float32).astype(ml_dtypes.bfloat16)

    ncls = int(lab.max()) + 1
    counts = np.bincount(lab, minlength=ncls)
    cum = np.concatenate([[0], np.cumsum(counts)])
    cnt = counts[lab].astype(np.float64)                  # incl. self
    num_pos = float(cnt.sum() - B)

    emb64 = emb.astype(np.float64)
    G = np.zeros((ncls, D), np.float64)
    np.add.at(G, lab, emb64)
    rds = (emb64 * G[lab]).sum(1) * TINV                  # sum_same s_ij (incl self)
    sii = (emb64 * emb64).sum(1) * TINV
    host_extra = float((sii - rds).sum())                 # sum_rows (s_ii - sum_{j!=i} s_ij)

    # device-exact e^{s_ii}: replicate the fp8 matmul's diagonal
    et64 = ET8.astype(np.float64)
    sii_dev = (et64 * et64).sum(0) * ESC                  # [B]
    eii_dev = np.exp(sii_dev).astype(np.float32)

    dr = dr_pack(ET8)                                     # [NKK, 128, 2, B]
    et_h = np.ascontiguousarray(np.stack([
        dr[0][:, :, 0:HALF], dr[1][:, :, 0:HALF],
        dr[0][:, :, HALF:], dr[1][:, :, HALF:]]))         # [4, 128, 2, HALF]

    bf = ml_dtypes.bfloat16
    in_maps = []
    for c in range(NCORES):
        sl = slice(c * BL, (c + 1) * BL)
        etwin = np.zeros((NMT, D, WIN), NP_F8)
        pack16 = np.zeros((128, NMT, WIN + 4), bf)
        pack16[:, :, :WIN] = bf(-1.0)                     # pad: matches no label
        for m in range(NMT):
            r0 = c * BL + m * 128
            c0 = int(cum[lab[r0]])
            c1 = int(cum[lab[r0 + 127] + 1])
            w = c1 - c0
            assert w <= WIN, f"window {w} exceeds {WIN}; rebuild with larger WIN"
            etwin[m, :, :w] = ET8[:, c0:c1]
            pack16[:, m, :w] = lab16[c0:c1][None, :]
            for p in range(128):
                pack16[p, m, r0 + p - c0] = bf(-2.0)      # poison self
            pack16[:, m, WIN] = lab16[r0:r0 + 128]
            pack16[:, m, WIN + 1] = (cnt[r0:r0 + 128] - 1.0).astype(bf)
            ehi = eii_dev[r0:r0 + 128].astype(bf)
            pack16[:, m, WIN + 2] = ehi
            pack16[:, m, WIN + 3] = (
                eii_dev[r0:r0 + 128] - ehi.astype(np.float32)).astype(bf)
        # pack8: [128, 2+NMT, 2, 2, WIN]; j=0,1 own-row lhsT; j=2+m window rhs
        pack8 = np.zeros((128, 2 + NMT, 2, 2, WIN), NP_F8)
        eltp = dr_pack(np.ascontiguousarray(ET8[:, sl]))  # [NKK, 128, 2, BL]
        for k in range(NKK):
            pack8[:, k] = eltp[k].reshape(128, 2, 2, WIN)
        etwin_packed = etwin.reshape(NMT, NKK, 2, 128, WIN).transpose(
            0, 3, 1, 2, 4)                                # [NMT, 128, 2, 2, WIN]
        for m in range(NMT):
            pack8[:, 2 + m] = etwin_packed[m]
        in_maps.append({
            "et": et_h,
            "pack8": pack8,
            "pack16": pack16,
        })
    return in_maps, num_pos, host_extra


def kernel(embeddings, labels):
    in_maps, num_pos, host_extra = _make_in_maps(embeddings, labels)
    if "nc" not in _CACHE:
        _CACHE["nc"] = _build_nc()
    nc = _CACHE["nc"]
    res = run_bass_kernel_spmd(nc, in_maps, list(range(NCORES)))
    total = sum(float(r["out"].sum()) for r in res.results) + host_extra
    return np.asarray(total / max(num_pos, 1.0), dtype=np.float32)


# revision 35
# speedup vs baseline: 1.0910x; 1.0233x over previous
"""Supervised contrastive loss on 8 Trainium2 NeuronCores.

Data-parallel over embedding rows (512 rows/core), label-sorted so each
128-row m-tile's same-label partners live in one <=256-column window.

Per core, per m-tile (ACT is the bottleneck engine; structure minimizes
ACT element count and instruction count):
  - window: 2 fp8 DoubleRow matmuls recompute the [128, 256] same-label
    sims; all 4 m-tiles' windows share one [128, 1024] PSUM tile drained
    by ONE Exp activate. Mask (label equality, self poisoned to -2) and
    masked row-sums run on DVE -> ssame (diagonal excluded).
  - dense: k-outer matmuls fill [128, 2048] PSUM halves (ping-pong);
    ONE Exp activate per half writes an fp16 tile; DVE row-reduces it
    (fp16 keeps the e^{s_ii}~e^10 diagonal accurate enough to subtract).
  - per-row loss, with cnt, e^{s_ii} and sum_j s_ij host-precomputed:
      denom = sall - ssame - e_ii ;  lnden = ln(denom)
      rowtot = (cnt-1)*lnden + sum_win ln(1 + mew/denom)
    (the host adds sum_rows (s_ii - sum_same s_ij) and divides by the
    exact positive count).
"""

import ml_dtypes
import numpy as np

import concourse.bass as bass
import concourse.bacc as bacc
import concourse.mybir as mybir
import concourse.tile as tile
from concourse.bass_utils import run_bass_kernel_spmd
from concourse.hw_specs import get_activation_tables

B = 4096          # total rows
D = 512           # embedding dim
NCORES = 8
BL = B // NCORES  # rows per core
NKK = 2           # DoubleRow k-tiles (256 contraction rows each)
NMT = BL // 128   # m-tiles per core
HALF = 2048       # dense column half (4 PSUM banks)
WIN = 256         # same-label column window per m-tile (data max is 223)
MMW = 512        # matmul output width (columns per matmul instruction)
TINV = 10.0       # 1 / temperature
F32 = mybir.dt.float32
F16 = mybir.dt.float16
BF16 = mybir.dt.bfloat16
F8 = mybir.dt.float8e4
NP_F8 = mybir.dt.np(F8)
SCALE = 16.0      # fp8 pre-scale; folded out via the Exp activation scale
ESC = TINV / (SCALE * SCALE)

_CACHE = {}


def _build_nc():
    nc = bacc.Bacc()
    # et packed as 4 blocks (k0,h0),(k1,h0),(k0,h1),(k1,h1): k-split lets the
    # k0 matmuls of a half start before its k1 block lands
    et = nc.dram_tensor("et", [NKK * 2, 128, 2, HALF], F8, kind="ExternalInput")
    # pack8: one full-line DMA for all small fp8 inputs.
    # j=0,1: own-row lhsT k-tiles; j=2..5: window rhs for m-tile j-2
    pack8 = nc.dram_tensor("pack8", [128, 2 + NMT, 2, 2, WIN], F8,
                           kind="ExternalInput")
    # pack16: window labels (self poisoned -2, pad -1) + per-row scalars
    # [..., WIN]=row label, [WIN+1]=cnt-1, [WIN+2]=eii_hi, [WIN+3]=eii_lo
    pack16 = nc.dram_tensor("pack16", [128, NMT, WIN + 4], BF16,
                            kind="ExternalInput")
    out = nc.dram_tensor("out", [128, NMT], F32, kind="ExternalOutput")

    AF = mybir.ActivationFunctionType
    OP = mybir.AluOpType
    DR = mybir.MatmulPerfMode.DoubleRow
    AX = mybir.AxisListType.X

    with tile.TileContext(nc) as tc:
        with (
            tc.tile_pool(name="const", bufs=1) as cpool,
            tc.tile_pool(name="psum", bufs=2, space=bass.MemorySpace.PSUM) as ppool,
            tc.tile_pool(name="expo", bufs=2) as epool,
            tc.tile_pool(name="win", bufs=1) as wpool,
            tc.tile_pool(name="small", bufs=1) as spool,
        ):
            # [128, half, kk2, 2, HALF]: each block DMA is one contiguous
            # 4 KiB line per partition
            etall = cpool.tile([128, 2, NKK, 2, HALF], F8, tag="etall",
                               name="etall")
            pk8 = cpool.tile([128, 2 + NMT, 2, 2, WIN], F8, tag="pk8",
                             name="pk8")
            pk16 = cpool.tile([128, NMT, WIN + 4], BF16, tag="pk16",
                              name="pk16")

            def lhsT(k, mt):
                # own-row weights for m-tile mt: [128, 2, 128]
                return pk8[:, k, :, mt // 2,
                           (mt % 2) * 128:(mt % 2) * 128 + 128]

            def rhs(k, h, c, w):
                # dense columns [h*HALF+c, +w): [128, 2, w]
                return etall[:, h, k, :, c:c + w]

            # DMA issue: big et blocks on the Act HWDGE ring (issued before
            # any activation work), packed small inputs on the SP ring.
            for j in range(2):
                nc.scalar.dma_start(etall[:, j // 2, j % 2], et[j])
            # one table load serves both Exp and Ln (avoids set thrash)
            tabs = list(get_activation_tables(nc.m.arch).keys())
            nc.scalar.add_instruction(mybir.InstLoadActFuncSet(
                name=nc.get_next_instruction_name(), ins=[], outs=[],
                act_func_set_id=tabs.index("natural_log_exp_and_others")))
            for j in range(2, NKK * 2):
                nc.scalar.dma_start(etall[:, j // 2, j % 2], et[j])
            nc.sync.dma_start(pk8[:], pack8[:])
            nc.sync.dma_start(pk16[:], pack16[:])

            # ---- window sims: one PSUM tile, one Exp ----
            wp = ppool.tile([128, HALF], F32, tag="ps", name="wpsum")
            for mt in range(NMT):
                for k in range(NKK):
                    nc.tensor.matmul(
                        wp[:, mt * WIN:(mt + 1) * WIN],
                        lhsT(k, mt), pk8[:, 2 + mt, k],
                        start=(k == 0), stop=(k == NKK - 1), perf_mode=DR)
            expw = wpool.tile([128, NMT, WIN], BF16, tag="expw", name="expw")
            nc.scalar.activation(expw[:], wp[:, 0:NMT * WIN], AF.Exp, scale=ESC)

            # per-row scalars to f32: [...,0]=row label, [...,1]=cnt-1
            scal = spool.tile([128, NMT, 2], F32, tag="scal", name="scal")
            nc.vector.tensor_scalar(scal[:], pk16[:, :, WIN:WIN + 2], 0.0,
                                    None, OP.add)
            # e^{s_ii} from its two bf16 halves
            eii = spool.tile([128, NMT], F32, tag="eii", name="eii")
            nc.vector.tensor_reduce(eii[:], pk16[:, :, WIN + 2:WIN + 4],
                                    AX, OP.add)

            maskw = wpool.tile([128, NMT, WIN], BF16, tag="maskw", name="maskw")
            for mt in range(NMT):
                nc.vector.tensor_scalar(
                    maskw[:, mt], pk16[:, mt, 0:WIN],
                    scal[:, mt, 0:1], None, OP.is_equal)
            mew = wpool.tile([128, NMT, WIN], BF16, tag="mew", name="mew")
            nc.vector.tensor_tensor(mew[:], expw[:], maskw[:], OP.mult)
            ssame = spool.tile([128, NMT], F32, tag="ssame", name="ssame")
            nc.vector.tensor_reduce(ssame[:], mew[:], AX, OP.add)

            # ---- dense + per-m-tile tail ----
            rsum = spool.tile([128, NMT, 2], F32, tag="rsum", name="rsum")
            sall = spool.tile([128, NMT], F32, tag="sall", name="sall")
            denom = spool.tile([128, NMT], F32, tag="denom", name="denom")
            inv = spool.tile([128, NMT], F32, tag="inv", name="inv")
            lnden = spool.tile([128, NMT], F32, tag="lnden", name="lnden")
            slog = spool.tile([128, NMT], F32, tag="slog", name="slog")
            rowt = spool.tile([128, NMT], F32, tag="rowt", name="rowt")
            lnp = wpool.tile([128, NMT, WIN], BF16, tag="lnp", name="lnp")

            for mt in range(NMT):
                for h in range(2):
                    P = ppool.tile([128, HALF], F32, tag="ps",
                                   name=f"ps{mt}_{h}")
                    for k in range(NKK):
                        w = lhsT(k, mt)
                        for j in range(HALF // MMW):
                            nc.tensor.matmul(
                                P[:, j * MMW:(j + 1) * MMW],
                                w, rhs(k, h, j * MMW, MMW),
                                start=(k == 0), stop=(k == NKK - 1),
                                perf_mode=DR)
                    if h == 0:
                        # h0 row-sum on DVE (fp16 keeps the e^{s_ii} diagonal
                        # accurate); hidden under the h1 chunk's work
                        E = epool.tile([128, HALF], F16, tag="expA",
                                       name=f"exp{mt}_{h}")
                        nc.scalar.activation(E[:], P[:], AF.Exp, scale=ESC)
                        nc.vector.tensor_reduce(rsum[:, mt, 0:1], E[:],
                                                AX, OP.add)
                    else:
                        E = epool.tile([128, HALF], BF16, tag="expB",
                                       name=f"exp{mt}_{h}")
                        nc.scalar.activation(E[:], P[:], AF.Exp, scale=ESC,
                                             accum_out=rsum[:, mt, 1:2])
                # tail for this m-tile (overlaps later m-tiles' dense work)
                nc.vector.tensor_reduce(sall[:, mt:mt + 1], rsum[:, mt],
                                        AX, OP.add)
                nc.vector.tensor_scalar(
                    denom[:, mt:mt + 1], sall[:, mt:mt + 1],
                    ssame[:, mt:mt + 1], eii[:, mt:mt + 1],
                    OP.subtract, OP.subtract)
                nc.vector.reciprocal(inv[:, mt:mt + 1], denom[:, mt:mt + 1])
                nc.scalar.activation(lnden[:, mt:mt + 1], denom[:, mt:mt + 1],
                                     AF.Ln)
                nc.scalar.activation(lnp[:, mt], mew[:, mt], AF.Ln,
                                     scale=inv[:, mt:mt + 1], bias=1.0)
                nc.vector.tensor_reduce(slog[:, mt:mt + 1], lnp[:, mt],
                                        AX, OP.add)
                nc.vector.tensor_scalar(
                    rowt[:, mt:mt + 1], lnden[:, mt:mt + 1],
                    scal[:, mt, 1:2], slog[:, mt:mt + 1],
                    OP.mult, OP.add)
            nc.scalar.dma_start(out[:], rowt[:])
    nc.compile()
    return nc


def _make_in_maps(embeddings, labels):
    """Host prep: label-sort, fp8 transposes, windows, per-row scalars.
    Returns (in_maps, num_pos, host_extra): host_extra is added to the
    device row-total sum before dividing by num_pos."""
    emb0 = np.ascontiguousarray(np.asarray(embeddings, dtype=np.float32))
    lab0 = np.asarray(labels).astype(np.int64)
    assert emb0.shape == (B, D) and lab0.shape == (B,)

    perm = np.argsort(lab0, kind="stable")
    emb = emb0[perm]
    lab = lab0[perm]

    ET = np.ascontiguousarray(emb.T)                      # [D, B] sorted cols
    ET8 = (ET * SCALE).astype(NP_F8)

    def dr_pack(a):
        # [D, X] -> [NKK, 128, 2, X] with d = kk*256 + ko*128 + ki
        X = a.shape[1]
        return np.ascontiguousarray(
            a.reshape(NKK, 2, 128, X).transpose(0, 2, 1, 3))

    lab16 = lab.astype(np.float32).astype(ml_dtypes.bfloat16)

    ncls = int(lab.max()) + 1
    counts = np.bincount(lab, minlength=ncls)
    cum = np.concatenate([[0], np.cumsum(counts)])
    cnt = counts[lab].astype(np.float64)                  # incl. self
    num_pos = float(cnt.sum() - B)

    emb64 = emb.astype(np.float64)
    G = np.zeros((ncls, D), np.float64)
    np.add.at(G, lab, emb64)
    rds = (emb64 * G[lab]).sum(1) * TINV                  # sum_same s_ij (incl self)
    sii = (emb64 * emb64).sum(1) * TINV
    host_extra = float((sii - rds).sum())                 # sum_rows (s_ii - sum_{j!=i} s_ij)

    # device-exact e^{s_ii}: replicate the fp8 matmul's diagonal
    et64 = ET8.astype(np.float64)
    sii_dev = (et64 * et64).sum(0) * ESC                  # [B]
    eii_dev = np.exp(sii_dev).astype(np.float32)

    dr = dr_pack(ET8)                                     # [NKK, 128, 2, B]
    et_h = np.ascontiguousarray(np.stack([
        dr[0][:, :, 0:HALF], dr[1][:, :, 0:HALF],
        dr[0][:, :, HALF:], dr[1][:, :, HALF:]]))         # [4, 128, 2, HALF]

    bf = ml_dtypes.bfloat16
    in_maps = []
    for c in range(NCORES):
        sl = slice(c * BL, (c + 1) * BL)
        etwin = np.zeros((NMT, D, WIN), NP_F8)
        pack16 = np.zeros((128, NMT, WIN + 4), bf)
        pack16[:, :, :WIN] = bf(-1.0)                     # pad: matches no label
        for m in range(NMT):
            r0 = c * BL + m * 128
            c0 = int(cum[lab[r0]])
            c1 = int(cum[lab[r0 + 127] + 1])
            w = c1 - c0
            assert w <= WIN, f"window {w} exceeds {WIN}; rebuild with larger WIN"
            etwin[m, :, :w] = ET8[:, c0:c1]
            pack16[:, m, :w] = lab16[c0:c1][None, :]
            for p in range(128):
                pack16[p, m, r0 + p - c0] = bf(-2.0)      # poison self
            pack16[:, m, WIN] = lab16[r0:r0 + 128]
            pack16[:, m, WIN + 1] = (cnt[r0:r0 + 128] - 1.0).astype(bf)
            ehi = eii_dev[r0:r0 + 128].astype(bf)
            pack16[:, m, WIN + 2] = ehi
            pack16[:, m, WIN + 3] = (
                eii_dev[r0:r0 + 128] - ehi.astype(np.float32)).astype(bf)
        # pack8: [128, 2+NMT, 2, 2, WIN]; j=0,1 own-row lhsT; j=2+m window rhs
        pack8 = np.zeros((128, 2 + NMT, 2, 2, WIN), NP_F8)
        eltp = dr_pack(np.ascontiguousarray(ET8[:, sl]))  # [NKK, 128, 2, BL]
        for k in range(NKK):
            pack8[:, k] = eltp[k].reshape(128, 2, 2, WIN)
        etwin_packed = etwin.reshape(NMT, NKK, 2, 128, WIN).transpose(
            0, 3, 1, 2, 4)                                # [NMT, 128, 2, 2, WIN]
        for m in range(NMT):
            pack8[:, 2 + m] = etwin_packed[m]
        in_maps.append({
            "et": et_h,
            "pack8": pack8,
            "pack16": pack16,
        })
    return in_maps, num_pos, host_extra


def kernel(embeddings, labels):
    in_maps, num_pos, host_extra = _make_in_maps(embeddings, labels)
    if "nc" not in _CACHE:
        _CACHE["nc"] = _build_nc()
    nc = _CACHE["nc"]
    res = run_bass_kernel_spmd(nc, in_maps, list(range(NCORES)))
    total = sum(float(r["out"].sum()) for r in res.results) + host_extra
    return np.asarray(total / max(num_pos, 1.0), dtype=np.float32)


# revision 36
# speedup vs baseline: 1.1374x; 1.0426x over previous
"""Supervised contrastive loss on 8 Trainium2 NeuronCores.

Data-parallel over embedding rows (512 rows/core), label-sorted so each
128-row m-tile's same-label partners live in one <=256-column window.

Per core, per m-tile (ACT is the bottleneck engine; structure minimizes
ACT element count and instruction count):
  - window: 2 fp8 DoubleRow matmuls recompute the [128, 256] same-label
    sims; all 4 m-tiles' windows share one [128, 1024] PSUM tile drained
    by ONE Exp activate. Mask (label equality, self poisoned to -2) and
    masked row-sums run on DVE -> ssame (diagonal excluded).
  - dense: k-outer matmuls fill [128, 2048] PSUM halves (ping-pong);
    ONE Exp activate per half writes an fp16 tile; DVE row-reduces it
    (fp16 keeps the e^{s_ii}~e^10 diagonal accurate enough to subtract).
  - per-row loss, with cnt, e^{s_ii} and sum_j s_ij host-precomputed:
      denom = sall - ssame - e_ii ;  lnden = ln(denom)
      rowtot = (cnt-1)*lnden + sum_win ln(1 + mew/denom)
    (the host adds sum_rows (s_ii - sum_same s_ij) and divides by the
    exact positive count).
"""

import ml_dtypes
import numpy as np

import concourse.bass as bass
import concourse.bacc as bacc
import concourse.mybir as mybir
import concourse.tile as tile
from concourse.bass_utils import run_bass_kernel_spmd
from concourse.hw_specs import get_activation_tables

B = 4096          # total rows
D = 512           # embedding dim
NCORES = 8
BL = B // NCORES  # rows per core
NKK = 2           # DoubleRow k-tiles (256 contraction rows each)
NMT = BL // 128   # m-tiles per core
HALF = 2048       # dense column half (4 PSUM banks)
WIN = 256         # same-label column window per m-tile (data max is 223)
MMW = 512        # matmul output width (columns per matmul instruction)
TINV = 10.0       # 1 / temperature
F32 = mybir.dt.float32
F16 = mybir.dt.float16
BF16 = mybir.dt.bfloat16
F8 = mybir.dt.float8e4
NP_F8 = mybir.dt.np(F8)
SCALE = 16.0      # fp8 pre-scale; folded out via the Exp activation scale
ESC = TINV / (SCALE * SCALE)

_CACHE = {}


def _build_nc():
    nc = bacc.Bacc()
    # et packed as 4 blocks (k0,h0),(k1,h0),(k0,h1),(k1,h1): k-split lets the
    # k0 matmuls of a half start before its k1 block lands
    et = nc.dram_tensor("et", [NKK * 2, 128, 2, HALF], F8, kind="ExternalInput")
    # pack8: one full-line DMA for all small fp8 inputs.
    # j=0,1: own-row lhsT k-tiles; j=2..5: window rhs for m-tile j-2
    pack8 = nc.dram_tensor("pack8", [128, 2 + NMT, 2, 2, WIN], F8,
                           kind="ExternalInput")
    # pack16: window labels (self poisoned -2, pad -1) + per-row scalars
    # [..., WIN]=row label, [WIN+1]=cnt-1, [WIN+2]=eii_hi, [WIN+3]=eii_lo
    pack16 = nc.dram_tensor("pack16", [128, NMT, WIN + 4], BF16,
                            kind="ExternalInput")
    out = nc.dram_tensor("out", [128, NMT], F32, kind="ExternalOutput")

    AF = mybir.ActivationFunctionType
    OP = mybir.AluOpType
    DR = mybir.MatmulPerfMode.DoubleRow
    AX = mybir.AxisListType.X

    with tile.TileContext(nc) as tc:
        with (
            tc.tile_pool(name="const", bufs=1) as cpool,
            tc.tile_pool(name="psum", bufs=2, space=bass.MemorySpace.PSUM) as ppool,
            tc.tile_pool(name="expo", bufs=2) as epool,
            tc.tile_pool(name="win", bufs=1) as wpool,
            tc.tile_pool(name="small", bufs=1) as spool,
        ):
            # [128, half, kk2, 2, HALF]: each block DMA is one contiguous
            # 4 KiB line per partition
            etall = cpool.tile([128, 2, NKK, 2, HALF], F8, tag="etall",
                               name="etall")
            pk8 = cpool.tile([128, 2 + NMT, 2, 2, WIN], F8, tag="pk8",
                             name="pk8")
            pk16 = cpool.tile([128, NMT, WIN + 4], BF16, tag="pk16",
                              name="pk16")

            def lhsT(k, mt):
                # own-row weights for m-tile mt: [128, 2, 128]
                return pk8[:, k, :, mt // 2,
                           (mt % 2) * 128:(mt % 2) * 128 + 128]

            def rhs(k, h, c, w):
                # dense columns [h*HALF+c, +w): [128, 2, w]
                return etall[:, h, k, :, c:c + w]

            # DMA issue: big et blocks on the Act HWDGE ring (issued before
            # any activation work), packed small inputs on the SP ring.
            for j in range(2):
                nc.scalar.dma_start(etall[:, j // 2, j % 2], et[j])
            # one table load serves both Exp and Ln (avoids set thrash)
            tabs = list(get_activation_tables(nc.m.arch).keys())
            nc.scalar.add_instruction(mybir.InstLoadActFuncSet(
                name=nc.get_next_instruction_name(), ins=[], outs=[],
                act_func_set_id=tabs.index("natural_log_exp_and_others")))
            for j in range(2, NKK * 2):
                nc.scalar.dma_start(etall[:, j // 2, j % 2], et[j])
            nc.sync.dma_start(pk8[:], pack8[:])
            nc.sync.dma_start(pk16[:], pack16[:])

            # ---- window sims: one PSUM tile, one Exp ----
            wp = ppool.tile([128, HALF], F32, tag="ps", name="wpsum")
            for mt in range(NMT):
                for k in range(NKK):
                    nc.tensor.matmul(
                        wp[:, mt * WIN:(mt + 1) * WIN],
                        lhsT(k, mt), pk8[:, 2 + mt, k],
                        start=(k == 0), stop=(k == NKK - 1), perf_mode=DR)
            expw = wpool.tile([128, NMT, WIN], BF16, tag="expw", name="expw")
            nc.scalar.activation(expw[:], wp[:, 0:NMT * WIN], AF.Exp, scale=ESC)

            # per-row scalars to f32: [...,0]=row label, [...,1]=cnt-1
            scal = spool.tile([128, NMT, 2], F32, tag="scal", name="scal")
            nc.vector.tensor_scalar(scal[:], pk16[:, :, WIN:WIN + 2], 0.0,
                                    None, OP.add)
            # e^{s_ii} from its two bf16 halves
            eii = spool.tile([128, NMT], F32, tag="eii", name="eii")
            nc.vector.tensor_reduce(eii[:], pk16[:, :, WIN + 2:WIN + 4],
                                    AX, OP.add)

            maskw = wpool.tile([128, NMT, WIN], BF16, tag="maskw", name="maskw")
            for mt in range(NMT):
                nc.vector.tensor_scalar(
                    maskw[:, mt], pk16[:, mt, 0:WIN],
                    scal[:, mt, 0:1], None, OP.is_equal)
            mew = wpool.tile([128, NMT, WIN], BF16, tag="mew", name="mew")
            nc.vector.tensor_tensor(mew[:], expw[:], maskw[:], OP.mult)
            ssame = spool.tile([128, NMT], F32, tag="ssame", name="ssame")
            nc.vector.tensor_reduce(ssame[:], mew[:], AX, OP.add)

            # ---- dense + per-m-tile tail ----
            rsum = spool.tile([128, NMT, 2], F32, tag="rsum", name="rsum")
            sall = spool.tile([128, NMT], F32, tag="sall", name="sall")
            denom = spool.tile([128, NMT], F32, tag="denom", name="denom")
            inv = spool.tile([128, NMT], F32, tag="inv", name="inv")
            lnden = spool.tile([128, NMT], F32, tag="lnden", name="lnden")
            slog = spool.tile([128, NMT], F32, tag="slog", name="slog")
            rowt = spool.tile([128, NMT], F32, tag="rowt", name="rowt")
            lnp = wpool.tile([128, NMT, WIN], BF16, tag="lnp", name="lnp")

            # h0 sweep first: consumes the et blocks that land first, so the
            # ACT stream never stalls on the later h1 blocks
            for mt in range(NMT):
                P = ppool.tile([128, HALF], F32, tag="ps", name=f"ps{mt}_0")
                for k in range(NKK):
                    w = lhsT(k, mt)
                    for j in range(HALF // MMW):
                        nc.tensor.matmul(
                            P[:, j * MMW:(j + 1) * MMW],
                            w, rhs(k, 0, j * MMW, MMW),
                            start=(k == 0), stop=(k == NKK - 1),
                            perf_mode=DR)
                # h0 row-sum on DVE (fp16 keeps the e^{s_ii} diagonal
                # accurate); hidden under later chunks' work
                E = epool.tile([128, HALF], F16, tag="expA",
                               name=f"exp{mt}_0")
                nc.scalar.activation(E[:], P[:], AF.Exp, scale=ESC)
                nc.vector.tensor_reduce(rsum[:, mt, 0:1], E[:], AX, OP.add)

            for mt in range(NMT):
                P = ppool.tile([128, HALF], F32, tag="ps", name=f"ps{mt}_1")
                for k in range(NKK):
                    w = lhsT(k, mt)
                    for j in range(HALF // MMW):
                        nc.tensor.matmul(
                            P[:, j * MMW:(j + 1) * MMW],
                            w, rhs(k, 1, j * MMW, MMW),
                            start=(k == 0), stop=(k == NKK - 1),
                            perf_mode=DR)
                E = epool.tile([128, HALF], BF16, tag="expB",
                               name=f"exp{mt}_1")
                nc.scalar.activation(E[:], P[:], AF.Exp, scale=ESC,
                                     accum_out=rsum[:, mt, 1:2])
                # tail for this m-tile (overlaps later m-tiles' dense work)
                nc.vector.tensor_reduce(sall[:, mt:mt + 1], rsum[:, mt],
                                        AX, OP.add)
                nc.vector.tensor_scalar(
                    denom[:, mt:mt + 1], sall[:, mt:mt + 1],
                    ssame[:, mt:mt + 1], eii[:, mt:mt + 1],
                    OP.subtract, OP.subtract)
                nc.vector.reciprocal(inv[:, mt:mt + 1], denom[:, mt:mt + 1])
                nc.scalar.activation(lnden[:, mt:mt + 1], denom[:, mt:mt + 1],
                                     AF.Ln)
                nc.scalar.activation(lnp[:, mt], mew[:, mt], AF.Ln,
                                     scale=inv[:, mt:mt + 1], bias=1.0)
                nc.vector.tensor_reduce(slog[:, mt:mt + 1], lnp[:, mt],
                                        AX, OP.add)
                nc.vector.tensor_scalar(
                    rowt[:, mt:mt + 1], lnden[:, mt:mt + 1],
                    scal[:, mt, 1:2], slog[:, mt:mt + 1],
                    OP.mult, OP.add)
            nc.scalar.dma_start(out[:], rowt[:])
    nc.compile()
    return nc


def _make_in_maps(embeddings, labels):
    """Host prep: label-sort, fp8 transposes, windows, per-row scalars.
    Returns (in_maps, num_pos, host_extra): host_extra is added to the
    device row-total sum before dividing by num_pos."""
    emb0 = np.ascontiguousarray(np.asarray(embeddings, dtype=np.float32))
    lab0 = np.asarray(labels).astype(np.int64)
    assert emb0.shape == (B, D) and lab0.shape == (B,)

    perm = np.argsort(lab0, kind="stable")
    emb = emb0[perm]
    lab = lab0[perm]

    ET = np.ascontiguousarray(emb.T)                      # [D, B] sorted cols
    ET8 = (ET * SCALE).astype(NP_F8)

    def dr_pack(a):
        # [D, X] -> [NKK, 128, 2, X] with d = kk*256 + ko*128 + ki
        X = a.shape[1]
        return np.ascontiguousarray(
            a.reshape(NKK, 2, 128, X).transpose(0, 2, 1, 3))

    lab16 = lab.astype(np.float32).astype(ml_dtypes.bfloat16)

    ncls = int(lab.max()) + 1
    counts = np.bincount(lab, minlength=ncls)
    cum = np.concatenate([[0], np.cumsum(counts)])
    cnt = counts[lab].astype(np.float64)                  # incl. self
    num_pos = float(cnt.sum() - B)

    emb64 = emb.astype(np.float64)
    G = np.zeros((ncls, D), np.float64)
    np.add.at(G, lab, emb64)
    rds = (emb64 * G[lab]).sum(1) * TINV                  # sum_same s_ij (incl self)
    sii = (emb64 * emb64).sum(1) * TINV
    host_extra = float((sii - rds).sum())                 # sum_rows (s_ii - sum_{j!=i} s_ij)

    # device-exact e^{s_ii}: replicate the fp8 matmul's diagonal
    et64 = ET8.astype(np.float64)
    sii_dev = (et64 * et64).sum(0) * ESC                  # [B]
    eii_dev = np.exp(sii_dev).astype(np.float32)

    dr = dr_pack(ET8)                                     # [NKK, 128, 2, B]
    et_h = np.ascontiguousarray(np.stack([
        dr[0][:, :, 0:HALF], dr[1][:, :, 0:HALF],
        dr[0][:, :, HALF:], dr[1][:, :, HALF:]]))         # [4, 128, 2, HALF]

    bf = ml_dtypes.bfloat16
    in_maps = []
    for c in range(NCORES):
        sl = slice(c * BL, (c + 1) * BL)
        etwin = np.zeros((NMT, D, WIN), NP_F8)
        pack16 = np.zeros((128, NMT, WIN + 4), bf)
        pack16[:, :, :WIN] = bf(-1.0)                     # pad: matches no label
        for m in range(NMT):
            r0 = c * BL + m * 128
            c0 = int(cum[lab[r0]])
            c1 = int(cum[lab[r0 + 127] + 1])
            w = c1 - c0
            assert w <= WIN, f"window {w} exceeds {WIN}; rebuild with larger WIN"
            etwin[m, :, :w] = ET8[:, c0:c1]
            pack16[:, m, :w] = lab16[c0:c1][None, :]
            for p in range(128):
                pack16[p, m, r0 + p - c0] = bf(-2.0)      # poison self
            pack16[:, m, WIN] = lab16[r0:r0 + 128]
            pack16[:, m, WIN + 1] = (cnt[r0:r0 + 128] - 1.0).astype(bf)
            ehi = eii_dev[r0:r0 + 128].astype(bf)
            pack16[:, m, WIN + 2] = ehi
            pack16[:, m, WIN + 3] = (
                eii_dev[r0:r0 + 128] - ehi.astype(np.float32)).astype(bf)
        # pack8: [128, 2+NMT, 2, 2, WIN]; j=0,1 own-row lhsT; j=2+m window rhs
        pack8 = np.zeros((128, 2 + NMT, 2, 2, WIN), NP_F8)
        eltp = dr_pack(np.ascontiguousarray(ET8[:, sl]))  # [NKK, 128, 2, BL]
        for k in range(NKK):
            pack8[:, k] = eltp[k].reshape(128, 2, 2, WIN)
        etwin_packed = etwin.reshape(NMT, NKK, 2, 128, WIN).transpose(
            0, 3, 1, 2, 4)                                # [NMT, 128, 2, 2, WIN]
        for m in range(NMT):
            pack8[:, 2 + m] = etwin_packed[m]
        in_maps.append({
            "et": et_h,
            "pack8": pack8,
            "pack16": pack16,
        })
    return in_maps, num_pos, host_extra


def kernel(embeddings, labels):
    in_maps, num_pos, host_extra = _make_in_maps(embeddings, labels)
    if "nc" not in _CACHE:
        _CACHE["nc"] = _build_nc()
    nc = _CACHE["nc"]
    res = run_bass_kernel_spmd(nc, in_maps, list(range(NCORES)))
    total = sum(float(r["out"].sum()) for r in res.results) + host_extra
    return np.asarray(total / max(num_pos, 1.0), dtype=np.float32)


# revision 37
# speedup vs baseline: 1.1594x; 1.0194x over previous
"""Supervised contrastive loss on 8 Trainium2 NeuronCores.

Data-parallel over embedding rows (512 rows/core), label-sorted so each
128-row m-tile's same-label partners live in one <=256-column window.

Per core, per m-tile (ACT is the bottleneck engine; structure minimizes
ACT element count and instruction count):
  - window: 2 fp8 DoubleRow matmuls recompute the [128, 256] same-label
    sims; all 4 m-tiles' windows share one [128, 1024] PSUM tile drained
    by ONE Exp activate. Mask (label equality, self poisoned to -2) and
    masked row-sums run on DVE -> ssame (diagonal excluded).
  - dense: k-outer matmuls fill [128, 2048] PSUM halves (ping-pong);
    ONE Exp activate per half writes an fp16 tile; DVE row-reduces it
    (fp16 keeps the e^{s_ii}~e^10 diagonal accurate enough to subtract).
  - per-row loss, with cnt, e^{s_ii} and sum_j s_ij host-precomputed:
      denom = sall - ssame - e_ii ;  lnden = ln(denom)
      rowtot = (cnt-1)*lnden + sum_win ln(1 + mew/denom)
    (the host adds sum_rows (s_ii - sum_same s_ij) and divides by the
    exact positive count).
"""

import ml_dtypes
import numpy as np

import concourse.bass as bass
import concourse.bacc as bacc
import concourse.mybir as mybir
import concourse.tile as tile
from concourse.bass_utils import run_bass_kernel_spmd
from concourse.hw_specs import get_activation_tables

B = 4096          # total rows
D = 512           # embedding dim
NCORES = 8
BL = B // NCORES  # rows per core
NKK = 2           # DoubleRow k-tiles (256 contraction rows each)
NMT = BL // 128   # m-tiles per core
HALF = 2048       # dense column half (4 PSUM banks)
WIN = 256         # same-label column window per m-tile (data max is 223)
MMW = 512        # matmul output width (columns per matmul instruction)
TINV = 10.0       # 1 / temperature
F32 = mybir.dt.float32
F16 = mybir.dt.float16
BF16 = mybir.dt.bfloat16
F8 = mybir.dt.float8e4
NP_F8 = mybir.dt.np(F8)
SCALE = 16.0      # fp8 pre-scale; folded out via the Exp activation scale
ESC = TINV / (SCALE * SCALE)

_CACHE = {}


def _build_nc():
    nc = bacc.Bacc()
    # et packed as 4 blocks (k0,h0),(k1,h0),(k0,h1),(k1,h1): k-split lets the
    # k0 matmuls of a half start before its k1 block lands
    et = nc.dram_tensor("et", [NKK * 2, 128, 2, HALF], F8, kind="ExternalInput")
    # pack8: one full-line DMA for all small fp8 inputs.
    # j=0,1: own-row lhsT k-tiles; j=2..5: window rhs for m-tile j-2
    pack8 = nc.dram_tensor("pack8", [128, 2 + NMT, 2, 2, WIN], F8,
                           kind="ExternalInput")
    # pack16: window labels (self poisoned -2, pad -1) + per-row scalars
    # [..., WIN]=row label, [WIN+1]=cnt-1, [WIN+2]=eii_hi, [WIN+3]=eii_lo
    pack16 = nc.dram_tensor("pack16", [128, NMT, WIN + 4], BF16,
                            kind="ExternalInput")
    out = nc.dram_tensor("out", [128, NMT], F32, kind="ExternalOutput")

    AF = mybir.ActivationFunctionType
    OP = mybir.AluOpType
    DR = mybir.MatmulPerfMode.DoubleRow
    AX = mybir.AxisListType.X

    with tile.TileContext(nc) as tc:
        with (
            tc.tile_pool(name="const", bufs=1) as cpool,
            tc.tile_pool(name="psum", bufs=2, space=bass.MemorySpace.PSUM) as ppool,
            tc.tile_pool(name="expo", bufs=3) as epool,
            tc.tile_pool(name="win", bufs=1) as wpool,
            tc.tile_pool(name="small", bufs=1) as spool,
        ):
            # [128, half, kk2, 2, HALF]: each block DMA is one contiguous
            # 4 KiB line per partition
            etall = cpool.tile([128, 2, NKK, 2, HALF], F8, tag="etall",
                               name="etall")
            pk8 = cpool.tile([128, 2 + NMT, 2, 2, WIN], F8, tag="pk8",
                             name="pk8")
            pk16 = cpool.tile([128, NMT, WIN + 4], BF16, tag="pk16",
                              name="pk16")

            def lhsT(k, mt):
                # own-row weights for m-tile mt: [128, 2, 128]
                return pk8[:, k, :, mt // 2,
                           (mt % 2) * 128:(mt % 2) * 128 + 128]

            def rhs(k, h, c, w):
                # dense columns [h*HALF+c, +w): [128, 2, w]
                return etall[:, h, k, :, c:c + w]

            # DMA issue: big et blocks on the Act HWDGE ring (issued before
            # any activation work), packed small inputs on the SP ring.
            for j in range(2):
                nc.scalar.dma_start(etall[:, j // 2, j % 2], et[j])
            # one table load serves both Exp and Ln (avoids set thrash)
            tabs = list(get_activation_tables(nc.m.arch).keys())
            nc.scalar.add_instruction(mybir.InstLoadActFuncSet(
                name=nc.get_next_instruction_name(), ins=[], outs=[],
                act_func_set_id=tabs.index("natural_log_exp_and_others")))
            for j in range(2, NKK * 2):
                nc.scalar.dma_start(etall[:, j // 2, j % 2], et[j])
            nc.sync.dma_start(pk8[:], pack8[:])
            nc.sync.dma_start(pk16[:], pack16[:])

            # ---- window sims: one PSUM tile, one Exp ----
            wp = ppool.tile([128, HALF], F32, tag="ps", name="wpsum")
            for mt in range(NMT):
                for k in range(NKK):
                    nc.tensor.matmul(
                        wp[:, mt * WIN:(mt + 1) * WIN],
                        lhsT(k, mt), pk8[:, 2 + mt, k],
                        start=(k == 0), stop=(k == NKK - 1), perf_mode=DR)
            expw = wpool.tile([128, NMT, WIN], BF16, tag="expw", name="expw")
            nc.scalar.activation(expw[:], wp[:, 0:NMT * WIN], AF.Exp, scale=ESC)

            # per-row scalars to f32: [...,0]=row label, [...,1]=cnt-1
            scal = spool.tile([128, NMT, 2], F32, tag="scal", name="scal")
            nc.vector.tensor_scalar(scal[:], pk16[:, :, WIN:WIN + 2], 0.0,
                                    None, OP.add)
            # e^{s_ii} from its two bf16 halves
            eii = spool.tile([128, NMT], F32, tag="eii", name="eii")
            nc.vector.tensor_reduce(eii[:], pk16[:, :, WIN + 2:WIN + 4],
                                    AX, OP.add)

            maskw = wpool.tile([128, NMT, WIN], BF16, tag="maskw", name="maskw")
            for mt in range(NMT):
                nc.vector.tensor_scalar(
                    maskw[:, mt], pk16[:, mt, 0:WIN],
                    scal[:, mt, 0:1], None, OP.is_equal)
            mew = wpool.tile([128, NMT, WIN], BF16, tag="mew", name="mew")
            nc.vector.tensor_tensor(mew[:], expw[:], maskw[:], OP.mult)
            ssame = spool.tile([128, NMT], F32, tag="ssame", name="ssame")
            nc.vector.tensor_reduce(ssame[:], mew[:], AX, OP.add)

            # ---- dense + per-m-tile tail ----
            rsum = spool.tile([128, NMT, 2], F32, tag="rsum", name="rsum")
            sall = spool.tile([128, NMT], F32, tag="sall", name="sall")
            denom = spool.tile([128, NMT], F32, tag="denom", name="denom")
            inv = spool.tile([128, NMT], F32, tag="inv", name="inv")
            lnden = spool.tile([128, NMT], F32, tag="lnden", name="lnden")
            slog = spool.tile([128, NMT], F32, tag="slog", name="slog")
            rowt = spool.tile([128, NMT], F32, tag="rowt", name="rowt")
            lnp = wpool.tile([128, NMT, WIN], BF16, tag="lnp", name="lnp")

            # h0 sweep first: consumes the et blocks that land first, so the
            # ACT stream never stalls on the later h1 blocks
            for mt in range(NMT):
                P = ppool.tile([128, HALF], F32, tag="ps", name=f"ps{mt}_0")
                for k in range(NKK):
                    w = lhsT(k, mt)
                    for j in range(HALF // MMW):
                        nc.tensor.matmul(
                            P[:, j * MMW:(j + 1) * MMW],
                            w, rhs(k, 0, j * MMW, MMW),
                            start=(k == 0), stop=(k == NKK - 1),
                            perf_mode=DR)
                # h0 row-sum on DVE (fp16 keeps the e^{s_ii} diagonal
                # accurate); hidden under later chunks' work
                E = epool.tile([128, HALF], F16, tag="expA",
                               name=f"exp{mt}_0")
                nc.scalar.activation(E[:], P[:], AF.Exp, scale=ESC)
                nc.vector.tensor_reduce(rsum[:, mt, 0:1], E[:], AX, OP.add)

            for mt in range(NMT):
                P = ppool.tile([128, HALF], F32, tag="ps", name=f"ps{mt}_1")
                for k in range(NKK):
                    w = lhsT(k, mt)
                    for j in range(HALF // MMW):
                        nc.tensor.matmul(
                            P[:, j * MMW:(j + 1) * MMW],
                            w, rhs(k, 1, j * MMW, MMW),
                            start=(k == 0), stop=(k == NKK - 1),
                            perf_mode=DR)
                E = epool.tile([128, HALF], BF16, tag="expB",
                               name=f"exp{mt}_1")
                nc.scalar.activation(E[:], P[:], AF.Exp, scale=ESC,
                                     accum_out=rsum[:, mt, 1:2])
                # tail for this m-tile (overlaps later m-tiles' dense work)
                nc.vector.tensor_reduce(sall[:, mt:mt + 1], rsum[:, mt],
                                        AX, OP.add)
                nc.vector.tensor_scalar(
                    denom[:, mt:mt + 1], sall[:, mt:mt + 1],
                    ssame[:, mt:mt + 1], eii[:, mt:mt + 1],
                    OP.subtract, OP.subtract)
                nc.vector.reciprocal(inv[:, mt:mt + 1], denom[:, mt:mt + 1])
                nc.scalar.activation(lnden[:, mt:mt + 1], denom[:, mt:mt + 1],
                                     AF.Ln)
                nc.scalar.activation(lnp[:, mt], mew[:, mt], AF.Ln,
                                     scale=inv[:, mt:mt + 1], bias=1.0)
                nc.vector.tensor_reduce(slog[:, mt:mt + 1], lnp[:, mt],
                                        AX, OP.add)
                nc.vector.tensor_scalar(
                    rowt[:, mt:mt + 1], lnden[:, mt:mt + 1],
                    scal[:, mt, 1:2], slog[:, mt:mt + 1],
                    OP.mult, OP.add)
            nc.scalar.dma_start(out[:], rowt[:])
    nc.compile()
    return nc


def _make_in_maps(embeddings, labels):
    """Host prep: label-sort, fp8 transposes, windows, per-row scalars.
    Returns (in_maps, num_pos, host_extra): host_extra is added to the
    device row-total sum before dividing by num_pos."""
    emb0 = np.ascontiguousarray(np.asarray(embeddings, dtype=np.float32))
    lab0 = np.asarray(labels).astype(np.int64)
    assert emb0.shape == (B, D) and lab0.shape == (B,)

    perm = np.argsort(lab0, kind="stable")
    emb = emb0[perm]
    lab = lab0[perm]

    ET = np.ascontiguousarray(emb.T)                      # [D, B] sorted cols
    ET8 = (ET * SCALE).astype(NP_F8)

    def dr_pack(a):
        # [D, X] -> [NKK, 128, 2, X] with d = kk*256 + ko*128 + ki
        X = a.shape[1]
        return np.ascontiguousarray(
            a.reshape(NKK, 2, 128, X).transpose(0, 2, 1, 3))

    lab16 = lab.astype(np.float32).astype(ml_dtypes.bfloat16)

    ncls = int(lab.max()) + 1
    counts = np.bincount(lab, minlength=ncls)
    cum = np.concatenate([[0], np.cumsum(counts)])
    cnt = counts[lab].astype(np.float64)                  # incl. self
    num_pos = float(cnt.sum() - B)

    emb64 = emb.astype(np.float64)
    G = np.zeros((ncls, D), np.float64)
    np.add.at(G, lab, emb64)
    rds = (emb64 * G[lab]).sum(1) * TINV                  # sum_same s_ij (incl self)
    sii = (emb64 * emb64).sum(1) * TINV
    host_extra = float((sii - rds).sum())                 # sum_rows (s_ii - sum_{j!=i} s_ij)

    # device-exact e^{s_ii}: replicate the fp8 matmul's diagonal
    et64 = ET8.astype(np.float64)
    sii_dev = (et64 * et64).sum(0) * ESC                  # [B]
    eii_dev = np.exp(sii_dev).astype(np.float32)

    dr = dr_pack(ET8)                                     # [NKK, 128, 2, B]
    et_h = np.ascontiguousarray(np.stack([
        dr[0][:, :, 0:HALF], dr[1][:, :, 0:HALF],
        dr[0][:, :, HALF:], dr[1][:, :, HALF:]]))         # [4, 128, 2, HALF]

    bf = ml_dtypes.bfloat16
    in_maps = []
    for c in range(NCORES):
        sl = slice(c * BL, (c + 1) * BL)
        etwin = np.zeros((NMT, D, WIN), NP_F8)
        pack16 = np.zeros((128, NMT, WIN + 4), bf)
        pack16[:, :, :WIN] = bf(-1.0)                     # pad: matches no label
        for m in range(NMT):
            r0 = c * BL + m * 128
            c0 = int(cum[lab[r0]])
            c1 = int(cum[lab[r0 + 127] + 1])
            w = c1 - c0
            assert w <= WIN, f"window {w} exceeds {WIN}; rebuild with larger WIN"
            etwin[m, :, :w] = ET8[:, c0:c1]
            pack16[:, m, :w] = lab16[c0:c1][None, :]
            for p in range(128):
                pack16[p, m, r0 + p - c0] = bf(-2.0)      # poison self
            pack16[:, m, WIN] = lab16[r0:r0 + 128]
            pack16[:, m, WIN + 1] = (cnt[r0:r0 + 128] - 1.0).astype(bf)
            ehi = eii_dev[r0:r0 + 128].astype(bf)
            pack16[:, m, WIN + 2] = ehi
            pack16[:, m, WIN + 3] = (
                eii_dev[r0:r0 + 128] - ehi.astype(np.float32)).astype(bf)
        # pack8: [128, 2+NMT, 2, 2, WIN]; j=0,1 own-row lhsT; j=2+m window rhs
        pack8 = np.zeros((128, 2 + NMT, 2, 2, WIN), NP_F8)
        eltp = dr_pack(np.ascontiguousarray(ET8[:, sl]))  # [NKK, 128, 2, BL]
        for k in range(NKK):
            pack8[:, k] = eltp[k].reshape(128, 2, 2, WIN)
        etwin_packed = etwin.reshape(NMT, NKK, 2, 128, WIN).transpose(
            0, 3, 1, 2, 4)                                # [NMT, 128, 2, 2, WIN]
        for m in range(NMT):
            pack8[:, 2 + m] = etwin_packed[m]
        in_maps.append({
            "et": et_h,
            "pack8": pack8,
            "pack16": pack16,
        })
    return in_maps, num_pos, host_extra


def kernel(embeddings, labels):
    in_maps, num_pos, host_extra = _make_in_maps(embeddings, labels)
    if "nc" not in _CACHE:
        _CACHE["nc"] = _build_nc()
    nc = _CACHE["nc"]
    res = run_bass_kernel_spmd(nc, in_maps, list(range(NCORES)))
    total = sum(float(r["out"].sum()) for r in res.results) + host_extra
    return np.asarray(total / max(num_pos, 1.0), dtype=np.float32)
